# revision 16
# baseline (speedup 1.0000x reference)
"""GAT layer kernel for Trainium2, 8 NeuronCores, data-parallel over R=b*s.

Self-contained: takes full inputs, returns full output.

v2 design (per core, RC=6 replicas):
  - Projection on PE: h_aug = x_r @ [W(c-major) | Ws | Wd]; h (bf16) plus the
    per-node a_src scalars are written into ONE HBM row per node:
    row n = [h r0..r5 (6*256 bf16, c-major) | a_src 24 bf16 | pad] (3328 B).
    a_dst scalars stay in SBUF ([125, nt, 24]).
  - Edge phase is chunked by dst-tile (125 dsts, all its edges, dst-sorted,
    padded to 128-slot tiles). Per chunk ONE dma_gather fetches, for every
    edge slot, the full row of its src node (~1800 descriptors, under the
    SWDGE FIFO limit).
  - z = a_src[src] (gathered) + a_dst[dst] (PE expand via transposed one-hot)
    p = exp(leaky_relu(z)) ; den = segment_sum(p) (PE one-hot);
    denrec = 0.25/den (node space - applied after aggregation, so no
    alpha normalization per edge is needed).
  - msg = hg * p (DVE, bf16, heads broadcast over c-major layout)
    num = segment_sum(msg) (PE one-hot, f32 PSUM, all 6 replicas per chunk)
  - out = sum_h denrec[d,h]*num[d,(c,h)] + bias  (node space, then DMA out).
"""

import math
import numpy as np
import ml_dtypes

B, S, N, F = 4, 12, 1000, 64
H, C = 4, 64
HC = H * C            # 256
R = B * S             # 48
NCORES = 8
RC = R // NCORES      # 6 replicas per core
NEG_SLOPE = 0.2
DTW = 125             # dst-tile width (8 tiles cover N=1000)
NDT = N // DTW        # 8
AC = RC * H           # 24 active scalar columns
ROWW = 1664           # h_hbm row width in bf16 (6*256 h + 24 as + pad) = 3328B

_CACHE = {}


# --------------------------------------------------------------------------
# host-side index preprocessing
# --------------------------------------------------------------------------
def _prep_edges(edge_index):
    src0 = np.asarray(edge_index[0], dtype=np.int64)
    dst0 = np.asarray(edge_index[1], dtype=np.int64)
    keep = src0 != dst0                      # PyG remove_self_loops + NEG_INF mask
    s_all = np.concatenate([src0[keep], np.arange(N, dtype=np.int64)])
    d_all = np.concatenate([dst0[keep], np.arange(N, dtype=np.int64)])
    order = np.argsort(d_all, kind="stable")
    s_all, d_all = s_all[order], d_all[order]

    # per dst-tile slot lists, each padded to a multiple of 128
    chunks = []
    for dt in range(NDT):
        lo, hi = dt * DTW, (dt + 1) * DTW
        m = (d_all >= lo) & (d_all < hi)
        ss, dd = s_all[m], d_all[m]
        cnt = len(ss)
        ntile = max(1, math.ceil(cnt / 128))
        pad = ntile * 128 - cnt
        ss = np.concatenate([ss, np.full(pad, 1000, np.int64)])   # pad -> row 1000
        dd = np.concatenate([dd, np.full(pad, lo, np.int64)])
        real = np.concatenate([np.ones(cnt, bool), np.zeros(pad, bool)])
        # one-hot [p, t, dlocal] and transposed [t, dlocal, p]
        oh = np.zeros((128, ntile, DTW), np.float32)
        for j in range(ntile * 128):
            if real[j]:
                oh[j % 128, j // 128, dd[j] - lo] = 1.0
        chunks.append(dict(ntile=ntile, src=ss, oh=oh.astype(ml_dtypes.bfloat16),
                           ohT=np.ascontiguousarray(
                               oh.transpose(2, 1, 0)).astype(ml_dtypes.bfloat16)))

    maxt = max(c["ntile"] for c in chunks)
    T = sum(c["ntile"] for c in chunks)
    # index tensor: per chunk, slots wrapped [16, slots/16], replicated to 128
    ihw = np.zeros((128, T * 8), np.int16)   # 128 slots = 8 idx columns
    oh_all = np.zeros((128, T, DTW), ml_dtypes.bfloat16)
    ohT_all = np.zeros((128, T, 128), ml_dtypes.bfloat16)
    t0 = 0
    for c in chunks:
        nt_, ss = c["ntile"], c["src"]
        ni = nt_ * 128
        a = np.zeros((16, ni // 16), np.int16)
        a[np.arange(ni) % 16, np.arange(ni) // 16] = ss.astype(np.int16)
        ihw[:, t0 * 8:(t0 + nt_) * 8] = np.tile(a, (8, 1))
        oh_all[:, t0:t0 + nt_, :] = c["oh"]
        ohT_all[:DTW, t0:t0 + nt_, :] = c["ohT"].transpose(0, 1, 2).reshape(
            DTW, nt_, 128)
        t0 += nt_
    tile_of = np.concatenate([[i] * c["ntile"] for i, c in enumerate(chunks)])
    return {
        "T": T, "maxt": maxt, "ntiles": [c["ntile"] for c in chunks],
        "oh": np.ascontiguousarray(oh_all.reshape(128, T * DTW)),
        "ohT": np.ascontiguousarray(ohT_all.reshape(128, T * 128)),
        "ih": ihw, "tile_of": tile_of,
    }


def _prep_weights(W, att_src, att_dst):
    W = np.asarray(W, np.float32)
    Ws = np.zeros((F, H), np.float32)
    Wd = np.zeros((F, H), np.float32)
    for h in range(H):
        Ws[:, h] = W[:, h * C:(h + 1) * C] @ np.asarray(att_src, np.float32)[h]
        Wd[:, h] = W[:, h * C:(h + 1) * C] @ np.asarray(att_dst, np.float32)[h]
    # c-major head interleave: device col c*4+h = W col h*64+c
    Wc = np.empty_like(W)
    for h in range(H):
        Wc[:, np.arange(C) * H + h] = W[:, h * C:(h + 1) * C]
    return np.concatenate([Wc, Ws, Wd], axis=1)      # [64, 264]


def _make_in_maps(x, W, att_src, att_dst, bias, ed):
    waug = _prep_weights(W, att_src, att_dst)
    bias_slab = np.tile(np.asarray(bias, np.float32)[None, :],
                        (128, RC)).reshape(128, RC * F)
    xr = np.ascontiguousarray(np.asarray(x, np.float32)).reshape(R, N, F)
    in_maps = []
    for cidx in range(NCORES):
        xc = xr[cidx * RC:(cidx + 1) * RC]
        xT = np.ascontiguousarray(xc.transpose(2, 0, 1).reshape(F, RC * N))
        in_maps.append({
            "xT": xT, "w_aug": waug, "oh": ed["oh"], "ohT": ed["ohT"],
            "ih": ed["ih"], "bias_slab": bias_slab,
        })
    return in_maps


# --------------------------------------------------------------------------
# device program
# --------------------------------------------------------------------------
def _build_program(ed):
    import concourse.bass as bass
    import concourse.mybir as mybir
    import concourse.tile as tile
    from concourse import bacc

    T, maxt = ed["T"], ed["maxt"]
    ntiles = ed["ntiles"]
    f32 = mybir.dt.float32
    bf16 = mybir.dt.bfloat16
    i16 = mybir.dt.int16
    Alu = mybir.AluOpType
    Act = mybir.ActivationFunctionType

    nc = bacc.Bacc("TRN2", target_bir_lowering=False, debug=False,
                   enable_asserts=False, num_devices=NCORES)

    xT_d = nc.dram_tensor("xT", [F, RC * N], f32, kind="ExternalInput").ap()
    waug_d = nc.dram_tensor("w_aug", [F, 264], f32, kind="ExternalInput").ap()
    oh_d = nc.dram_tensor("oh", [128, T * DTW], bf16, kind="ExternalInput").ap()
    ohT_d = nc.dram_tensor("ohT", [128, T * 128], bf16, kind="ExternalInput").ap()
    ih_d = nc.dram_tensor("ih", [128, T * 8], i16, kind="ExternalInput").ap()
    bias_d = nc.dram_tensor("bias_slab", [128, RC * F], f32, kind="ExternalInput").ap()
    out_d = nc.dram_tensor("out", [RC, N, F], f32, kind="ExternalOutput").ap()

    with tile.TileContext(nc) as tc:
        with (
            tc.tile_pool(name="const", bufs=1) as constp,
            tc.tile_pool(name="dram", bufs=1, space="DRAM") as dramp,
            tc.tile_pool(name="stage", bufs=3) as stagep,
            tc.tile_pool(name="edge", bufs=2) as edgep,
            tc.tile_pool(name="big", bufs=2) as bigp,
            tc.tile_pool(name="fin", bufs=2) as finp,
            tc.tile_pool(name="ppsum", bufs=2, space="PSUM") as ppsum,
            tc.tile_pool(name="dpsum", bufs=1, space="PSUM") as dpsum,
            tc.tile_pool(name="npsum", bufs=1, space="PSUM") as npsum,
        ):
            h_hbm = dramp.tile([N + 1, ROWW], bf16)

            # ---- constants ----
            waug = constp.tile([F, 264], f32)
            nc.sync.dma_start(waug[:], waug_d)
            oh = constp.tile([128, T, DTW], bf16)
            nc.sync.dma_start(oh[:], oh_d.rearrange("p (t d) -> p t d", d=DTW))
            ohT = constp.tile([128, T, 128], bf16)
            nc.sync.dma_start(ohT[:], ohT_d.rearrange("p (t e) -> p t e", e=128))
            ih = constp.tile([128, T * 8], i16)
            nc.sync.dma_start(ih[:], ih_d)
            bias_sl = constp.tile([128, RC, F], f32)
            nc.sync.dma_start(bias_sl[:], bias_d.rearrange("p (r f) -> p r f", f=F))

            # pad row 1000: h-part zeros, as-part -1000 => p == 0 for pad slots
            padrow = constp.tile([1, ROWW], bf16)
            nc.vector.memset(padrow[:], 0.0)
            nc.vector.memset(padrow[:, RC * HC:RC * HC + AC], -1000.0)
            nc.sync.dma_start(h_hbm[N:N + 1, :], padrow[:])

            # ---- phase A: projection; fills h_hbm and ad_sb ----
            ad_sb = constp.tile([DTW, NDT, AC], bf16)
            for r in range(RC):
                for nt in range(NDT):
                    n0 = nt * DTW
                    xt = stagep.tile([F, DTW], f32, tag="xt")
                    nc.sync.dma_start(xt[:], xT_d[:, r * N + n0: r * N + n0 + DTW])
                    ps = ppsum.tile([DTW, 264], f32, tag="proj")
                    nc.tensor.matmul(out=ps[:], lhsT=xt[:],
                                     rhs=waug[:], start=True, stop=True)
                    hsb = stagep.tile([DTW, HC + 8], bf16, tag="hsb")
                    nc.scalar.copy(out=hsb[:], in_=ps[:])
                    # h block (c-major cols) for this replica
                    nc.sync.dma_start(h_hbm[n0:n0 + DTW, r * HC:(r + 1) * HC],
                                      hsb[:, 0:HC])
                    # a_src scalars -> h_hbm tail cols (bf16)
                    nc.sync.dma_start(
                        h_hbm[n0:n0 + DTW, RC * HC + 4 * r:RC * HC + 4 * r + 4],
                        hsb[:, HC:HC + 4])
                    # a_dst scalars stay on-chip
                    nc.vector.tensor_copy(out=ad_sb[:, nt, 4 * r:4 * r + 4],
                                          in_=ps[:, HC + 4:HC + 8])

            # ---- per dst-tile chunks ----
            t0 = 0
            for dt in range(NDT):
                nt_ = ntiles[dt]
                ni = nt_ * 128
                assert ni <= 2032, "gather exceeds SWDGE FIFO; split needed"
                hg = bigp.tile([128, maxt, ROWW], bf16, tag="big")
                nc.gpsimd.dma_gather(
                    out_ap=hg[:, 0:nt_, :], in_ap=h_hbm[:],
                    idxs_ap=ih[:, t0 * 8:(t0 + nt_) * 8],
                    num_idxs=ni, num_idxs_reg=ni, elem_size=ROWW,
                    single_packet=False)

                # ad expand: [128(e), nt_*24] psum via transposed one-hot
                eps = dpsum.tile([128, maxt, AC], f32, tag="expand")
                for t in range(nt_):
                    nc.tensor.matmul(out=eps[:, t, :], lhsT=ohT[0:DTW, t0 + t, :],
                                     rhs=ad_sb[:, dt, :], start=True, stop=True)
                # z = as + ad ; leaky relu ; exp
                z = edgep.tile([128, maxt, AC], f32, tag="z")
                nc.vector.tensor_tensor(
                    out=z[:, 0:nt_, :],
                    in0=hg[:, 0:nt_, RC * HC:RC * HC + AC],
                    in1=eps[:, 0:nt_, :], op=Alu.add)
                nc.vector.scalar_tensor_tensor(
                    out=z[:, 0:nt_, :], in0=z[:, 0:nt_, :], scalar=NEG_SLOPE,
                    in1=z[:, 0:nt_, :], op0=Alu.mult, op1=Alu.max)
                p_bf = edgep.tile([128, maxt, AC], bf16, tag="p")
                nc.scalar.activation(out=p_bf[:, 0:nt_, :], in_=z[:, 0:nt_, :],
                                     func=Act.Exp)

                # den = segsum(p); denrec = 0.25/den
                den_ps = dpsum.tile([DTW, AC], f32, tag="den")
                for t in range(nt_):
                    nc.tensor.matmul(out=den_ps[:], lhsT=oh[:, t0 + t, :],
                                     rhs=p_bf[:, t, :],
                                     start=(t == 0), stop=(t == nt_ - 1))
                denrec = stagep.tile([DTW, AC], f32, tag="denrec")
                nc.vector.reciprocal(out=denrec[:], in_=den_ps[:])
                nc.vector.tensor_scalar_mul(denrec[:], denrec[:], 0.25)

                # apply p to all replicas in one DVE op (heads bcast, c-major)
                hgall = hg[:, 0:nt_, 0:RC * HC].rearrange(
                    "p t (r c h) -> p t r c h", h=H, c=C)
                pball = p_bf[:, 0:nt_, :].rearrange(
                    "p t (r o h) -> p t r o h", h=H, o=1).to_broadcast(
                    [128, nt_, RC, C, H])
                nc.vector.tensor_tensor(out=hgall, in0=hgall, in1=pball,
                                        op=Alu.mult)
                # segment-sum, 512-wide (bank-sized) across the r-contiguous cols
                nps = npsum.tile([DTW, RC, HC], f32, tag="num")
                npsf = nps[:].rearrange("d r e -> d (r e)")
                for k in range(3):
                    for t in range(nt_):
                        nc.tensor.matmul(out=npsf[:, 512 * k:512 * (k + 1)],
                                         lhsT=oh[:, t0 + t, :],
                                         rhs=hg[:, t, 512 * k:512 * (k + 1)],
                                         start=(t == 0), stop=(t == nt_ - 1))

                # finalize: numn = num * denrec (heads bcast), head-sum, bias
                numn = finp.tile([DTW, RC, HC], f32, tag="numn")
                drb = denrec[:].rearrange("d (r o h) -> d r o h", h=H, o=1
                                          ).to_broadcast([DTW, RC, C, H])
                nc.vector.tensor_tensor(
                    out=numn[:].rearrange("d r (c h) -> d r c h", h=H),
                    in0=nps[:].rearrange("d r (c h) -> d r c h", h=H),
                    in1=drb, op=Alu.mult)
                n4 = numn[:].rearrange("d r (c h) -> d r c h", h=H)
                t1 = finp.tile([DTW, RC, C], f32, tag="t1")
                t2 = finp.tile([DTW, RC, C], f32, tag="t2")
                ob = finp.tile([DTW, RC, C], f32, tag="ob")
                nc.vector.tensor_tensor(out=t1[:], in0=n4[:, :, :, 0],
                                        in1=n4[:, :, :, 1], op=Alu.add)
                nc.vector.tensor_tensor(out=t2[:], in0=n4[:, :, :, 2],
                                        in1=n4[:, :, :, 3], op=Alu.add)
                nc.vector.tensor_tensor(out=t1[:], in0=t1[:], in1=t2[:], op=Alu.add)
                nc.vector.tensor_tensor(out=ob[:], in0=t1[:],
                                        in1=bias_sl[0:DTW, :, :], op=Alu.add)
                nc.sync.dma_start(
                    out_d[:, dt * DTW:(dt + 1) * DTW, :].rearrange(
                        "r d f -> d r f"), ob[:])
                t0 += nt_

    nc.compile()
    return nc


# --------------------------------------------------------------------------
# public entry point
# --------------------------------------------------------------------------
def kernel(x, edge_index, W, att_src, att_dst, bias):
    key = hash(np.asarray(edge_index).tobytes())
    if key not in _CACHE:
        ed = _prep_edges(edge_index)
        _CACHE[key] = (_build_program(ed), ed)
    nc, ed = _CACHE[key]

    in_maps = _make_in_maps(x, W, att_src, att_dst, bias, ed)
    from concourse import bass_utils
    res = bass_utils.run_bass_kernel_spmd(nc, in_maps, core_ids=list(range(NCORES)))
    outs = [res.results[c]["out"] for c in range(NCORES)]
    out = np.concatenate(outs, axis=0).reshape(B, S, N, F).astype(np.float32)
    return out
